# revision 37
# baseline (speedup 1.0000x reference)
"""Trainium2 Bass kernel for the BAN (bilinear attention network) problem.

Math (per batch b, eval mode):
    hq = emb[he_ques] @ Wq + bq                  [NQ, H]
    hk = emb[he_kg]   @ Wk + bk                  [NK, H]
    logits[g,q,k] = sum_d hq[q,d] Watt[d,g] hk[k,d]   (+ batt[g], cancels in
                                                       the joint softmax)
    att = softmax over flattened (q,k) per (b,g)
    pooled[g,d] = sum_{q,k} hq[q,d] att[g,q,k] hk[k,d]
    out = pooled.flat @ Wout + bout;  sim = out @ glove.T;  log_softmax(sim)

Distribution: pure data parallel over batch, 8 samples per core on 8 cores.
All weights replicated. No collectives.

v5 design notes:
  - ALL matmul operands bf16 (single-pass PE ~0.42ns/row vs f32r two-pass).
    PSUM accumulation fp32. Scale-rel error ~1.7e-3 vs the 2e-2 gate.
  - emb host-cast bf16, padded to 384 cols (col 300 = ones bias trick);
    gathered X rows XBAR-transposed (one DMA per gather tile, Sync queue).
  - hkT computed by matmul (the D5 critical path never depends on a DMA
    transpose); token-major hk derived on the PE (transpose+copy, ~1us a
    pair) because each XBAR transpose costs ~1.3us of hwdge sequencer
    time, and sharing DMA completion semaphores across queues with the
    big weight streams caused false-satisfied waits (a real data race).
  - hqw (hq * Watt) batched for all samples inside phase C, layout
    [d, m, b, g, q] so D5's rhs slice is contiguous.
  - Z-reduction (D7) runs after the pooling matmuls so its tiny dependent
    matmul never stalls the PE; single fused pooled scale per sample.
  - wout/glove streams split into chunks emitted across the pair loop:
    one 4.9MB DMA monopolized the DMA path for ~18us and serialized
    unrelated transfers behind it.
  - Startup: identity built on-chip; K-pair-0 gathers before Q gathers;
    PE order warm -> D2(pair0) -> C -> samples.
  - Tail: no max shift (sim is O(+-5)); sim resident in PSUM; exp/reduce
    pipelined per chunk; final (sim - lnZ) split DVE/Act; lnZ via Ln(1/Z).
"""

import sys

if "/opt/trn_rl_repo" not in sys.path:
    sys.path.insert(0, "/opt/trn_rl_repo")

import numpy as np

import concourse.bass as bass
import concourse.mybir as mybir
import concourse.tile as tile
from concourse import bacc
from concourse.bass_utils import run_bass_kernel_spmd

F32 = mybir.dt.float32
BF16 = mybir.dt.bfloat16
I32 = mybir.dt.int32
AX = mybir.AxisListType
OP = mybir.AluOpType
AF = mybir.ActivationFunctionType

N_CORES = 8
VOCAB = 20000
E = 300          # word embedding size
EA = 384         # padded: col 300 = ones (bias trick), 301.. = zeros
H = 1024         # hidden
G = 8            # heads
N_OUT = 300
N_ANS = 4000
B, NQ, NK = 64, 32, 256
BL = B // N_CORES            # 8 samples per core
TQ = BL * NQ                 # 256 q tokens per core
TK = BL * NK                 # 2048 k tokens per core
TQ_TILES = TQ // 128         # 2
TK_TILES = TK // 128         # 16
DT = H // 128                # 8 d-tiles
N_CHUNKS = (128, 128, N_OUT - 256)   # (128, 128, 44) rows of the 300-dim
NA_CH = 8                    # sim computed in 8 chunks of 500
NA_W = N_ANS // NA_CH        # 500
NWOUT = G * DT               # 64 Wout k-tiles


def build_kernel():
    nc = bacc.Bacc("TRN2", target_bir_lowering=False, debug=False,
                   num_devices=N_CORES)

    # ---- DRAM I/O ----
    emb_d = nc.dram_tensor("emb", [VOCAB, EA], BF16, kind="ExternalInput").ap()
    idxq_d = nc.dram_tensor("idx_q", [128, TQ_TILES], I32, kind="ExternalInput").ap()
    idxk_d = nc.dram_tensor("idx_k", [128, TK_TILES], I32, kind="ExternalInput").ap()
    wq_d = nc.dram_tensor("wq", [EA, H], BF16, kind="ExternalInput").ap()
    wk_d = nc.dram_tensor("wk", [EA, H], BF16, kind="ExternalInput").ap()
    watt_d = nc.dram_tensor("watt", [128, DT, G], BF16, kind="ExternalInput").ap()
    wout_d = nc.dram_tensor("wout", [G * H, N_OUT], BF16, kind="ExternalInput").ap()
    bout_d = nc.dram_tensor("bout", [BL, N_OUT], F32, kind="ExternalInput").ap()
    glovet_d = nc.dram_tensor("glovet", [N_OUT, N_ANS], BF16,
                              kind="ExternalInput").ap()
    out_d = nc.dram_tensor("out", [BL, N_ANS], F32, kind="ExternalOutput").ap()
    warm_d = nc.dram_tensor("warm", [1, 128], F32, kind="ExternalOutput").ap()

    with tile.TileContext(nc) as tc:
        import contextlib

        with contextlib.ExitStack() as ctx:
            consts = ctx.enter_context(tc.tile_pool(name="consts", bufs=1))
            actx = contextlib.ExitStack()
            hqw_pool = actx.enter_context(tc.tile_pool(name="hqwp", bufs=1))
            xrow_p = actx.enter_context(tc.tile_pool(name="xrow", bufs=6))
            xkt_p = actx.enter_context(tc.tile_pool(name="xkt", bufs=2))
            hkt_p = actx.enter_context(tc.tile_pool(name="hkt", bufs=2))
            hk_p = actx.enter_context(tc.tile_pool(name="hk", bufs=2))
            et_p = actx.enter_context(tc.tile_pool(name="et", bufs=2))
            v_p = actx.enter_context(tc.tile_pool(name="v", bufs=2))
            vr_p = actx.enter_context(tc.tile_pool(name="vr", bufs=3))
            zz_p = actx.enter_context(tc.tile_pool(name="zz", bufs=2))
            zn_p = actx.enter_context(tc.tile_pool(name="zn", bufs=3))
            mm_p = actx.enter_context(tc.tile_pool(name="mm", bufs=3, space="PSUM"))
            tp_p = actx.enter_context(tc.tile_pool(name="tp", bufs=1, space="PSUM"))
            lg_p = actx.enter_context(tc.tile_pool(name="lg", bufs=1, space="PSUM"))
            up_p = actx.enter_context(tc.tile_pool(name="up", bufs=3, space="PSUM"))

            # ---- on-chip constants (no DMA round-trip) ----
            ident = consts.tile([128, 128], BF16, tag="ident")
            nc.gpsimd.memset(ident[:], 1.0)
            nc.gpsimd.affine_select(
                out=ident[:], in_=ident[:], pattern=[[-1, 128]], base=0,
                channel_multiplier=1, compare_op=OP.is_equal, fill=0.0,
            )
            ones_sb = consts.tile([128, 1], F32, tag="ones")
            nc.gpsimd.memset(ones_sb[:], 1.0)
            wz = consts.tile([128, 512], BF16, tag="wz")
            nc.vector.memset(wz[:], 0.0)

            # ---- critical input DMAs ----
            idxk_sb = consts.tile([128, TK_TILES], I32, tag="idxk")
            nc.sync.dma_start(idxk_sb[:], idxk_d)
            idxq_sb = consts.tile([128, TQ_TILES], I32, tag="idxq")
            nc.sync.dma_start(idxq_sb[:], idxq_d)
            wk_sb = consts.tile([128, 3, H], BF16, tag="wk")
            nc.sync.dma_start(
                wk_sb[:], wk_d.rearrange("(c p) h -> p c h", p=128))
            wq_sb = consts.tile([128, 3, H], BF16, tag="wq")
            nc.sync.dma_start(
                wq_sb[:], wq_d.rearrange("(c p) h -> p c h", p=128))
            watt_sb = consts.tile([128, DT, G], BF16, tag="watt")
            nc.sync.dma_start(watt_sb[:], watt_d)

            # ---- gathers: K pair 0 first (longer downstream chain) ----
            xrow_tiles = {}

            def gather(idx_sb, col):
                xrow = xrow_p.tile([128, EA], BF16, tag="xrow")
                nc.gpsimd.indirect_dma_start(
                    out=xrow[:],
                    out_offset=None,
                    in_=emb_d,
                    in_offset=bass.IndirectOffsetOnAxis(
                        ap=idx_sb[:, col : col + 1], axis=0
                    ),
                )
                return xrow

            def transpose_x(xrow, dst, dst_col):
                nc.sync.dma_start_transpose(
                    dst[:, :, dst_col * 128 : (dst_col + 1) * 128], xrow[:]
                )

            xqT = consts.tile([128, 3, TQ], BF16, tag="xqT")
            xkT0 = xkt_p.tile([128, 3, 512], BF16, tag="xkT")
            for t in range(4):
                xrow_tiles[("k", t)] = gather(idxk_sb, t)
            for t in range(TQ_TILES):
                xrow_tiles[("q", t)] = gather(idxq_sb, t)
            for t in range(4):
                transpose_x(xrow_tiles.pop(("k", t)), xkT0, t)
            for t in range(TQ_TILES):
                transpose_x(xrow_tiles.pop(("q", t)), xqT, t)

            # PE warmup: back-to-back N=512 bf16 matmuls push the HAM clock
            # ramp while the gathers land.
            wps = mm_p.tile([128, 512], F32, tag="mm")
            for _ in range(40):
                nc.tensor.matmul(wps[:], lhsT=ident[:], rhs=wz[:],
                                 start=True, stop=True)
            warm_sb = consts.tile([1, 128], F32, tag="warm")
            nc.vector.tensor_copy(warm_sb[:], wps[:1, :128])
            nc.sync.dma_start(warm_d, warm_sb[:])

            def project_k_pair(xkT):
                """hkT [d, k-pair] by matmul (so D5 never waits on a DMA
                transpose), then token-major hk via PE transposes, lagged
                one d-tile so each hkT copy has landed before its
                transpose reads it."""
                hkT = hkt_p.tile([128, DT, 512], BF16, tag="hkT")
                hk = hk_p.tile([128, 4, H], BF16, tag="hk")

                def emit_d2(m):
                    ps = mm_p.tile([128, 512], F32, tag="mm")
                    for c in range(3):
                        nc.tensor.matmul(
                            ps[:],
                            lhsT=wk_sb[:, c, m * 128 : (m + 1) * 128],
                            rhs=xkT[:, c, :],
                            start=(c == 0),
                            stop=(c == 2),
                        )
                    if m % 2 == 0:
                        nc.scalar.activation(out=hkT[:, m, :], in_=ps[:],
                                             func=AF.Copy)
                    else:
                        nc.vector.tensor_copy(out=hkT[:, m, :], in_=ps[:])

                def emit_transpose(m):
                    pt = tp_p.tile([128, 512], BF16, tag="mmT")
                    for t in range(4):
                        nc.tensor.transpose(
                            pt[:, t * 128 : (t + 1) * 128],
                            hkT[:, m, t * 128 : (t + 1) * 128],
                            ident[:],
                        )
                    nc.scalar.activation(
                        out=hk[:, :, m * 128 : (m + 1) * 128],
                        in_=pt[:].rearrange("p (t f) -> p t f", t=4),
                        func=AF.Copy)

                emit_d2(0)
                for m in range(1, DT):
                    emit_d2(m)
                    emit_transpose(m - 1)
                emit_transpose(DT - 1)
                return hk, hkT

            # ---- D2 pair 0 (before phase C so the PE rides the K path) ----
            hk_cur, hkT_cur = project_k_pair(xkT0)

            # ---- phase C: hqT + batched hqw, interleaved per d-tile ----
            # hqw layout [d, m, b, g, q]: D5's rhs slice [c, b] contiguous.
            hqT = consts.tile([128, DT, TQ], BF16, tag="hqT")
            hqw = hqw_pool.tile([128, DT, BL, G, NQ], BF16, tag="hqw")
            for m in range(DT):
                ps = mm_p.tile([128, 512], F32, tag="mm")
                for c in range(3):
                    nc.tensor.matmul(
                        ps[:, :TQ],
                        lhsT=wq_sb[:, c, m * 128 : (m + 1) * 128],
                        rhs=xqT[:, c, :],
                        start=(c == 0),
                        stop=(c == 2),
                    )
                nc.scalar.activation(out=hqT[:, m, :], in_=ps[:, :TQ],
                                     func=AF.Copy)
                nc.vector.tensor_tensor(
                    out=hqw[:, m],
                    in0=hqT[:, m, :].rearrange("p (b q) -> p b q", b=BL)[
                        :, :, None, :].to_broadcast([128, BL, G, NQ]),
                    in1=watt_sb[:, m, None, :, None].to_broadcast(
                        [128, BL, G, NQ]),
                    op=OP.mult,
                )

            # deferred weight streams, chunked so no single transfer
            # monopolizes the DMA path or skews semaphore completion order
            wout_sb = consts.tile([128, NWOUT, N_OUT], BF16, tag="wout")
            glove_sb = consts.tile([128, 3, N_ANS], BF16, tag="glove")
            bout_sb = consts.tile([BL, N_OUT], F32, tag="bout")

            def emit_weight_chunk(step):
                if step < 8:        # wout: 8 chunks of 8 k-tiles
                    lo = step * 8
                    nc.sync.dma_start(
                        wout_sb[:, lo : lo + 8, :],
                        wout_d[lo * 128 : (lo + 8) * 128].rearrange(
                            "(t p) n -> p t n", p=128))
                elif step < 10:     # glove rows 0..255 in 2 chunks
                    c = step - 8
                    nc.sync.dma_start(
                        glove_sb[:, c, :],
                        glovet_d[c * 128 : (c + 1) * 128])
                elif step == 10:    # glove rows 256..299
                    nc.sync.dma_start(glove_sb[: N_OUT - 256, 2, :],
                                      glovet_d[2 * 128 : N_OUT])
                elif step == 11:
                    nc.sync.dma_start(bout_sb[:], bout_d)

            poT = consts.tile([128, DT, G, BL], BF16, tag="poT")
            wstep = 0

            # ---- phase D: attention, two samples per pair ----
            for p in range(BL // 2):
                hk, hkT = hk_cur, hkT_cur
                xkT_next = None
                if p < 3:
                    xkT_next = xkt_p.tile([128, 3, 512], BF16, tag="xkT")
                    for t in range(4):
                        xrow_tiles[("k", t)] = gather(idxk_sb, (p + 1) * 4 + t)

                for bi in range(2):
                    b = p * 2 + bi

                    # D5: logits.T [k, (g,q)] in PSUM: [128, 2, 256]
                    ps_l = lg_p.tile([128, 512], F32, tag="lg")
                    for kt in range(2):
                        for c in range(DT):
                            nc.tensor.matmul(
                                ps_l[:, kt * 256 : (kt + 1) * 256],
                                lhsT=hkT[
                                    :, c,
                                    bi * 256 + kt * 128 : bi * 256 + (kt + 1) * 128,
                                ],
                                rhs=hqw[:, c, b],
                                start=(c == 0),
                                stop=(c == DT - 1),
                            )

                    # D6: E = exp(logits) bf16 (one op), zz sums (one op)
                    et = et_p.tile([128, 2, G * NQ], BF16, tag="et")
                    zz = zz_p.tile([128, 2, G], F32, tag="zz")
                    nc.scalar.activation(
                        out=et[:], in_=ps_l[:], func=AF.Exp)
                    nc.vector.tensor_reduce(
                        out=zz[:],
                        in_=et[:].rearrange("p t (g q) -> p t g q", g=G),
                        axis=AX.X,
                        op=OP.add,
                    )

                    # D8: u = hk.T @ E per 2 d-tiles; v = u * hq; vr = sum_q
                    vr_all = vr_p.tile([128, DT, G], F32, tag="vr")
                    for mp in range(4):
                        ps_u = up_p.tile([128, 512], F32, tag="up")
                        for mi in range(2):
                            m = mp * 2 + mi
                            for kt in range(2):
                                nc.tensor.matmul(
                                    ps_u[:, mi * 256 : (mi + 1) * 256],
                                    lhsT=hk[:, bi * 2 + kt, m * 128 : (m + 1) * 128],
                                    rhs=et[:, kt, :],
                                    start=(kt == 0),
                                    stop=(kt == 1),
                                )
                        v = v_p.tile([128, 2, G, NQ], BF16, tag="v")
                        nc.vector.tensor_tensor(
                            out=v[:],
                            in0=ps_u[:].rearrange("p (m g q) -> p m g q", m=2, g=G),
                            in1=hqT[
                                :, mp * 2 : mp * 2 + 2, None, b * NQ : (b + 1) * NQ
                            ].to_broadcast([128, 2, G, NQ]),
                            op=OP.mult,
                        )
                        nc.vector.tensor_reduce(
                            out=vr_all[:, mp * 2 : mp * 2 + 2, :], in_=v[:],
                            axis=AX.X, op=OP.add,
                        )

                    # D7 (late so the PE never waits on it): Z_g over
                    # k-partitions, then one fused pooled scale.
                    ps_z = mm_p.tile([128, 512], F32, tag="mm")
                    for kt in range(2):
                        nc.tensor.matmul(
                            ps_z[:1, :G],
                            lhsT=ones_sb[:],
                            rhs=zz[:, kt, :],
                            start=(kt == 0),
                            stop=(kt == 1),
                        )
                    zinv = zn_p.tile([1, G], F32, tag="zinv")
                    nc.vector.reciprocal(zinv[:1, :], ps_z[:1, :G])
                    zbro = zn_p.tile([128, G], F32, tag="zbro")
                    nc.gpsimd.partition_broadcast(zbro[:], zinv[:1, :], channels=128)
                    with nc.allow_low_precision(reason="bf16 pooled"):
                        nc.vector.tensor_tensor(
                            out=poT[:, :, :, b],
                            in0=vr_all[:],
                            in1=zbro[:, None, :].to_broadcast([128, DT, G]),
                            op=OP.mult,
                        )

                    if bi == 0:
                        if p < 3:
                            # next pair's X transposes mid-pair: their
                            # gathers land about now, and D2(p+1) needs
                            # xkT well before the pair ends
                            for t in range(4):
                                transpose_x(xrow_tiles.pop(("k", t)),
                                            xkT_next, t)
                        emit_weight_chunk(wstep); wstep += 1
                        emit_weight_chunk(wstep); wstep += 1

                emit_weight_chunk(wstep); wstep += 1
                if p < 3:
                    hk_cur, hkT_cur = project_k_pair(xkT_next)

            # attention pools (incl. all PSUM) are dead now
            actx.close()
            fctx = contextlib.ExitStack()
            fo_p = fctx.enter_context(tc.tile_pool(name="fo", bufs=2, space="PSUM"))

            # ---- phase F: out [8, 300] = pooled_flat @ Wout + bout ----
            ps_o = fo_p.tile([128, 512], F32, tag="fo")
            for g in range(G):
                for m in range(DT):
                    t = g * DT + m
                    nc.tensor.matmul(
                        ps_o[:BL, :N_OUT],
                        lhsT=poT[:, m, g, :],
                        rhs=wout_sb[:, t, :],
                        start=(t == 0),
                        stop=(t == NWOUT - 1),
                    )
            out_sb = consts.tile([BL, N_OUT], BF16, tag="out_sb")
            with nc.allow_low_precision(reason="bf16 out"):
                nc.vector.tensor_tensor(
                    out=out_sb[:], in0=ps_o[:BL, :N_OUT], in1=bout_sb[:], op=OP.add
                )

            # ---- phase G: sim + log_softmax (no max shift; sim is O(+-5)) --
            outT = consts.tile([128, 3, BL], BF16, tag="outT")
            for c, rows in enumerate(N_CHUNKS):
                psT = fo_p.tile([128, 128], BF16, tag="foT")
                nc.tensor.transpose(
                    psT[:rows, :BL],
                    out_sb[:, c * 128 : c * 128 + rows],
                    ident[:BL, :BL],
                )
                nc.scalar.activation(out=outT[:rows, c, :], in_=psT[:rows, :BL],
                                     func=AF.Copy)

            zs8 = consts.tile([BL, NA_CH], F32, tag="zs8")
            zs = consts.tile([BL, 1], F32, tag="zs")
            zsi = consts.tile([BL, 1], F32, tag="zsi")
            nlnz = consts.tile([BL, 1], F32, tag="nlnz")
            final_sb = consts.tile([BL, N_ANS], F32, tag="final")

            simp_tiles = []
            esc_p = ctx.enter_context(tc.tile_pool(name="esc", bufs=2))
            fctx.close()  # free F/outT PSUM banks before claiming all 8
            sim_p = ctx.enter_context(tc.tile_pool(name="simp", bufs=NA_CH,
                                                   space="PSUM"))
            for a in range(NA_CH):
                ps_s = sim_p.tile([128, NA_W], F32, tag="simp")
                for c, rows in enumerate(N_CHUNKS):
                    nc.tensor.matmul(
                        ps_s[:BL, :],
                        lhsT=outT[:rows, c, :],
                        rhs=glove_sb[:rows, c, a * NA_W : (a + 1) * NA_W],
                        start=(c == 0),
                        stop=(c == 2),
                    )
                esc = esc_p.tile([BL, NA_W], BF16, tag="esc")
                nc.scalar.activation(out=esc[:], in_=ps_s[:BL, :], func=AF.Exp)
                nc.vector.tensor_reduce(
                    out=zs8[:, a : a + 1], in_=esc[:], axis=AX.X, op=OP.add
                )
                simp_tiles.append(ps_s)

            nc.vector.tensor_reduce(out=zs[:], in_=zs8[:], axis=AX.X, op=OP.add)
            nc.vector.reciprocal(zsi[:], zs[:])
            nc.scalar.activation(out=nlnz[:], in_=zsi[:], func=AF.Ln)
            # final = sim - lnZ, 4 chunks on DVE + 4 on Act, then 2 DMAs
            for a in range(NA_CH):
                span = slice(a * NA_W, (a + 1) * NA_W)
                if a % 2 == 0:
                    nc.vector.tensor_scalar(
                        out=final_sb[:, span], in0=simp_tiles[a][:BL, :],
                        scalar1=nlnz[:], scalar2=None,
                        op0=OP.add,
                    )
                else:
                    nc.scalar.activation(
                        out=final_sb[:, span], in_=simp_tiles[a][:BL, :],
                        func=AF.Identity, bias=nlnz[:],
                    )
                if a == 3:
                    nc.sync.dma_start(out_d[:, : 4 * NA_W], final_sb[:, : 4 * NA_W])
            nc.sync.dma_start(out_d[:, 4 * NA_W :], final_sb[:, 4 * NA_W :])

    nc.compile()
    return nc


_NC = None


def _get_nc():
    global _NC
    if _NC is None:
        _NC = build_kernel()
    return _NC


def make_in_maps(inputs):
    import ml_dtypes

    bf = ml_dtypes.bfloat16
    he_q = np.asarray(inputs["he_ques"]).astype(np.int32)   # [64, 32]
    he_k = np.asarray(inputs["he_kg"]).astype(np.int32)     # [64, 256]
    emb0 = np.asarray(inputs["emb"], dtype=np.float32)
    emb = np.zeros((VOCAB, EA), dtype=bf)
    emb[:, :E] = emb0.astype(bf)
    emb[:, E] = np.ones((), dtype=bf)                       # bias column
    wq = np.zeros((EA, H), dtype=bf)
    wq[:E] = np.asarray(inputs["Wq"], np.float32).astype(bf)
    wq[E] = np.asarray(inputs["bq"], np.float32).astype(bf)
    wk = np.zeros((EA, H), dtype=bf)
    wk[:E] = np.asarray(inputs["Wk"], np.float32).astype(bf)
    wk[E] = np.asarray(inputs["bk"], np.float32).astype(bf)
    watt = np.ascontiguousarray(
        np.asarray(inputs["Watt"], np.float32).reshape(DT, 128, G)
        .transpose(1, 0, 2)).astype(bf)                     # [128, DT, G]
    wout = np.ascontiguousarray(
        np.asarray(inputs["Wout"], np.float32)).astype(bf)
    bout = np.ascontiguousarray(
        np.broadcast_to(np.asarray(inputs["bout"], np.float32), (BL, N_OUT)))
    glovet = np.ascontiguousarray(
        np.asarray(inputs["glove_cands"], np.float32).T).astype(bf)  # [300,4000]

    in_maps = []
    for i in range(N_CORES):
        iq = he_q[i * BL : (i + 1) * BL].reshape(-1)        # [256]
        ik = he_k[i * BL : (i + 1) * BL].reshape(-1)        # [2048]
        in_maps.append({
            "emb": emb,
            "idx_q": np.ascontiguousarray(iq.reshape(TQ_TILES, 128).T),
            "idx_k": np.ascontiguousarray(ik.reshape(TK_TILES, 128).T),
            "wq": wq,
            "wk": wk,
            "watt": watt,
            "wout": wout,
            "bout": bout,
            "glovet": glovet,
        })
    return in_maps


def kernel(**inputs) -> np.ndarray:
    nc = _get_nc()
    in_maps = make_in_maps(inputs)
    res = run_bass_kernel_spmd(nc, in_maps, list(range(N_CORES)))
    return np.concatenate(
        [np.asarray(res.results[i]["out"], np.float32) for i in range(N_CORES)],
        axis=0,
    )


# revision 40
# speedup vs baseline: 1.0005x; 1.0005x over previous
"""Trainium2 Bass kernel for the BAN (bilinear attention network) problem.

Math (per batch b, eval mode):
    hq = emb[he_ques] @ Wq + bq                  [NQ, H]
    hk = emb[he_kg]   @ Wk + bk                  [NK, H]
    logits[g,q,k] = sum_d hq[q,d] Watt[d,g] hk[k,d]   (+ batt[g], cancels in
                                                       the joint softmax)
    att = softmax over flattened (q,k) per (b,g)
    pooled[g,d] = sum_{q,k} hq[q,d] att[g,q,k] hk[k,d]
    out = pooled.flat @ Wout + bout;  sim = out @ glove.T;  log_softmax(sim)

Distribution: pure data parallel over batch, 8 samples per core on 8 cores.
All weights replicated. No collectives.

v5 design notes:
  - ALL matmul operands bf16 (single-pass PE ~0.42ns/row vs f32r two-pass).
    PSUM accumulation fp32. Scale-rel error ~1.7e-3 vs the 2e-2 gate.
  - emb host-cast bf16, padded to 384 cols (col 300 = ones bias trick);
    gathered X rows XBAR-transposed (one DMA per gather tile, Sync queue).
  - hkT computed by matmul (the D5 critical path never depends on a DMA
    transpose); token-major hk derived on the PE (transpose+copy, ~1us a
    pair) because each XBAR transpose costs ~1.3us of hwdge sequencer
    time, and sharing DMA completion semaphores across queues with the
    big weight streams caused false-satisfied waits (a real data race).
  - hqw (hq * Watt) batched for all samples inside phase C, layout
    [d, m, b, g, q] so D5's rhs slice is contiguous.
  - Z-reduction (D7) runs after the pooling matmuls so its tiny dependent
    matmul never stalls the PE; single fused pooled scale per sample.
  - wout/glove streams split into chunks emitted across the pair loop:
    one 4.9MB DMA monopolized the DMA path for ~18us and serialized
    unrelated transfers behind it.
  - Startup: identity built on-chip; K-pair-0 gathers before Q gathers;
    PE order warm -> D2(pair0) -> C -> samples.
  - Tail: no max shift (sim is O(+-5)); sim resident in PSUM; exp/reduce
    pipelined per chunk; final (sim - lnZ) split DVE/Act; lnZ via Ln(1/Z).
"""

import sys

if "/opt/trn_rl_repo" not in sys.path:
    sys.path.insert(0, "/opt/trn_rl_repo")

import numpy as np

import concourse.bass as bass
import concourse.mybir as mybir
import concourse.tile as tile
from concourse import bacc
from concourse.bass_utils import run_bass_kernel_spmd

F32 = mybir.dt.float32
BF16 = mybir.dt.bfloat16
I32 = mybir.dt.int32
AX = mybir.AxisListType
OP = mybir.AluOpType
AF = mybir.ActivationFunctionType

N_CORES = 8
VOCAB = 20000
E = 300          # word embedding size
EA = 384         # padded: col 300 = ones (bias trick), 301.. = zeros
H = 1024         # hidden
G = 8            # heads
N_OUT = 300
N_ANS = 4000
B, NQ, NK = 64, 32, 256
BL = B // N_CORES            # 8 samples per core
TQ = BL * NQ                 # 256 q tokens per core
TK = BL * NK                 # 2048 k tokens per core
TQ_TILES = TQ // 128         # 2
TK_TILES = TK // 128         # 16
DT = H // 128                # 8 d-tiles
N_CHUNKS = (128, 128, N_OUT - 256)   # (128, 128, 44) rows of the 300-dim
NA_CH = 8                    # sim computed in 8 chunks of 500
NA_W = N_ANS // NA_CH        # 500
NWOUT = G * DT               # 64 Wout k-tiles


def build_kernel():
    nc = bacc.Bacc("TRN2", target_bir_lowering=False, debug=False,
                   num_devices=N_CORES)

    # ---- DRAM I/O ----
    emb_d = nc.dram_tensor("emb", [VOCAB, EA], BF16, kind="ExternalInput").ap()
    idxq_d = nc.dram_tensor("idx_q", [128, TQ_TILES], I32, kind="ExternalInput").ap()
    idxk_d = nc.dram_tensor("idx_k", [128, TK_TILES], I32, kind="ExternalInput").ap()
    wq_d = nc.dram_tensor("wq", [EA, H], BF16, kind="ExternalInput").ap()
    wk_d = nc.dram_tensor("wk", [EA, H], BF16, kind="ExternalInput").ap()
    watt_d = nc.dram_tensor("watt", [128, DT, G], BF16, kind="ExternalInput").ap()
    wout_d = nc.dram_tensor("wout", [G * H, N_OUT], BF16, kind="ExternalInput").ap()
    bout_d = nc.dram_tensor("bout", [BL, N_OUT], F32, kind="ExternalInput").ap()
    glovet_d = nc.dram_tensor("glovet", [N_OUT, N_ANS], BF16,
                              kind="ExternalInput").ap()
    out_d = nc.dram_tensor("out", [BL, N_ANS], F32, kind="ExternalOutput").ap()
    warm_d = nc.dram_tensor("warm", [1, 128], F32, kind="ExternalOutput").ap()

    with tile.TileContext(nc) as tc:
        import contextlib

        with contextlib.ExitStack() as ctx:
            consts = ctx.enter_context(tc.tile_pool(name="consts", bufs=1))
            actx = contextlib.ExitStack()
            hqw_pool = actx.enter_context(tc.tile_pool(name="hqwp", bufs=1))
            xrow_p = actx.enter_context(tc.tile_pool(name="xrow", bufs=6))
            xkt_p = actx.enter_context(tc.tile_pool(name="xkt", bufs=2))
            hkt_p = actx.enter_context(tc.tile_pool(name="hkt", bufs=2))
            hk_p = actx.enter_context(tc.tile_pool(name="hk", bufs=2))
            et_p = actx.enter_context(tc.tile_pool(name="et", bufs=2))
            v_p = actx.enter_context(tc.tile_pool(name="v", bufs=2))
            vr_p = actx.enter_context(tc.tile_pool(name="vr", bufs=3))
            zz_p = actx.enter_context(tc.tile_pool(name="zz", bufs=2))
            zn_p = actx.enter_context(tc.tile_pool(name="zn", bufs=3))
            mm_p = actx.enter_context(tc.tile_pool(name="mm", bufs=3, space="PSUM"))
            tp_p = actx.enter_context(tc.tile_pool(name="tp", bufs=2, space="PSUM"))
            lg_p = actx.enter_context(tc.tile_pool(name="lg", bufs=1, space="PSUM"))
            up_p = actx.enter_context(tc.tile_pool(name="up", bufs=2, space="PSUM"))

            # ---- on-chip constants (no DMA round-trip) ----
            ident = consts.tile([128, 128], BF16, tag="ident")
            nc.gpsimd.memset(ident[:], 1.0)
            nc.gpsimd.affine_select(
                out=ident[:], in_=ident[:], pattern=[[-1, 128]], base=0,
                channel_multiplier=1, compare_op=OP.is_equal, fill=0.0,
            )
            ones_sb = consts.tile([128, 1], F32, tag="ones")
            nc.gpsimd.memset(ones_sb[:], 1.0)
            wz = consts.tile([128, 512], BF16, tag="wz")
            nc.vector.memset(wz[:], 0.0)

            # ---- critical input DMAs ----
            idxk_sb = consts.tile([128, TK_TILES], I32, tag="idxk")
            nc.sync.dma_start(idxk_sb[:], idxk_d)
            idxq_sb = consts.tile([128, TQ_TILES], I32, tag="idxq")
            nc.sync.dma_start(idxq_sb[:], idxq_d)
            wk_sb = consts.tile([128, 3, H], BF16, tag="wk")
            nc.sync.dma_start(
                wk_sb[:], wk_d.rearrange("(c p) h -> p c h", p=128))
            wq_sb = consts.tile([128, 3, H], BF16, tag="wq")
            nc.sync.dma_start(
                wq_sb[:], wq_d.rearrange("(c p) h -> p c h", p=128))
            watt_sb = consts.tile([128, DT, G], BF16, tag="watt")
            nc.sync.dma_start(watt_sb[:], watt_d)

            # ---- gathers: K pair 0 first (longer downstream chain) ----
            xrow_tiles = {}

            def gather(idx_sb, col):
                xrow = xrow_p.tile([128, EA], BF16, tag="xrow")
                nc.gpsimd.indirect_dma_start(
                    out=xrow[:],
                    out_offset=None,
                    in_=emb_d,
                    in_offset=bass.IndirectOffsetOnAxis(
                        ap=idx_sb[:, col : col + 1], axis=0
                    ),
                )
                return xrow

            def transpose_x(xrow, dst, dst_col):
                nc.sync.dma_start_transpose(
                    dst[:, :, dst_col * 128 : (dst_col + 1) * 128], xrow[:]
                )

            xqT = consts.tile([128, 3, TQ], BF16, tag="xqT")
            xkT0 = xkt_p.tile([128, 3, 512], BF16, tag="xkT")
            for t in range(4):
                xrow_tiles[("k", t)] = gather(idxk_sb, t)
            for t in range(TQ_TILES):
                xrow_tiles[("q", t)] = gather(idxq_sb, t)
            for t in range(4):
                transpose_x(xrow_tiles.pop(("k", t)), xkT0, t)
            for t in range(TQ_TILES):
                transpose_x(xrow_tiles.pop(("q", t)), xqT, t)

            # PE warmup: back-to-back N=512 bf16 matmuls push the HAM clock
            # ramp while the gathers land.
            wps = mm_p.tile([128, 512], F32, tag="mm")
            for _ in range(26):
                nc.tensor.matmul(wps[:], lhsT=ident[:], rhs=wz[:],
                                 start=True, stop=True)
            warm_sb = consts.tile([1, 128], F32, tag="warm")
            nc.vector.tensor_copy(warm_sb[:], wps[:1, :128])
            nc.sync.dma_start(warm_d, warm_sb[:])

            def project_k_pair(xkT):
                """hkT [d, k-pair] by matmul (so D5 never waits on a DMA
                transpose), then token-major hk via PE transposes, lagged
                one d-tile so each hkT copy has landed before its
                transpose reads it."""
                hkT = hkt_p.tile([128, DT, 512], BF16, tag="hkT")
                hk = hk_p.tile([128, 4, H], BF16, tag="hk")

                def emit_d2(m):
                    ps = mm_p.tile([128, 512], F32, tag="mm")
                    for c in range(3):
                        nc.tensor.matmul(
                            ps[:],
                            lhsT=wk_sb[:, c, m * 128 : (m + 1) * 128],
                            rhs=xkT[:, c, :],
                            start=(c == 0),
                            stop=(c == 2),
                        )
                    if m % 2 == 0:
                        nc.scalar.activation(out=hkT[:, m, :], in_=ps[:],
                                             func=AF.Copy)
                    else:
                        nc.vector.tensor_copy(out=hkT[:, m, :], in_=ps[:])

                def emit_transpose(m):
                    pt = tp_p.tile([128, 512], BF16, tag="mmT")
                    for t in range(4):
                        nc.tensor.transpose(
                            pt[:, t * 128 : (t + 1) * 128],
                            hkT[:, m, t * 128 : (t + 1) * 128],
                            ident[:],
                        )
                    nc.scalar.activation(
                        out=hk[:, :, m * 128 : (m + 1) * 128],
                        in_=pt[:].rearrange("p (t f) -> p t f", t=4),
                        func=AF.Copy)

                emit_d2(0)
                for m in range(1, DT):
                    emit_d2(m)
                    emit_transpose(m - 1)
                emit_transpose(DT - 1)
                return hk, hkT

            # ---- D2 pair 0 (before phase C so the PE rides the K path) ----
            hk_cur, hkT_cur = project_k_pair(xkT0)

            # ---- phase C: hqT + batched hqw, interleaved per d-tile ----
            # hqw layout [d, m, b, g, q]: D5's rhs slice [c, b] contiguous.
            # Two half-batch tiles (samples 0-3 / 4-7): dependency tracking
            # is tile-granular, so pair 0's D5 only waits on the low half.
            hqT = consts.tile([128, DT, TQ], BF16, tag="hqT")
            hqw_lo = hqw_pool.tile([128, DT, 4, G, NQ], BF16, tag="hqw_lo")
            hqw_hi = hqw_pool.tile([128, DT, 4, G, NQ], BF16, tag="hqw_hi")
            for m in range(DT):
                ps = mm_p.tile([128, 512], F32, tag="mm")
                for c in range(3):
                    nc.tensor.matmul(
                        ps[:, :TQ],
                        lhsT=wq_sb[:, c, m * 128 : (m + 1) * 128],
                        rhs=xqT[:, c, :],
                        start=(c == 0),
                        stop=(c == 2),
                    )
                nc.scalar.activation(out=hqT[:, m, :], in_=ps[:, :TQ],
                                     func=AF.Copy)
                for half, hw in ((0, hqw_lo), (1, hqw_hi)):
                    nc.vector.tensor_tensor(
                        out=hw[:, m],
                        in0=hqT[:, m, half * 128 : (half + 1) * 128]
                        .rearrange("p (b q) -> p b q", b=4)[
                            :, :, None, :].to_broadcast([128, 4, G, NQ]),
                        in1=watt_sb[:, m, None, :, None].to_broadcast(
                            [128, 4, G, NQ]),
                        op=OP.mult,
                    )

            # deferred weight streams, chunked so no single transfer
            # monopolizes the DMA path or skews semaphore completion order
            wout_sb = consts.tile([128, NWOUT, N_OUT], BF16, tag="wout")
            glove_sb = consts.tile([128, 3, N_ANS], BF16, tag="glove")
            bout_sb = consts.tile([BL, N_OUT], F32, tag="bout")

            def emit_weight_chunk(step):
                if step < 8:        # wout: 8 chunks of 8 k-tiles
                    lo = step * 8
                    nc.sync.dma_start(
                        wout_sb[:, lo : lo + 8, :],
                        wout_d[lo * 128 : (lo + 8) * 128].rearrange(
                            "(t p) n -> p t n", p=128))
                elif step < 10:     # glove rows 0..255 in 2 chunks
                    c = step - 8
                    nc.sync.dma_start(
                        glove_sb[:, c, :],
                        glovet_d[c * 128 : (c + 1) * 128])
                elif step == 10:    # glove rows 256..299
                    nc.sync.dma_start(glove_sb[: N_OUT - 256, 2, :],
                                      glovet_d[2 * 128 : N_OUT])
                elif step == 11:
                    nc.sync.dma_start(bout_sb[:], bout_d)

            poT = consts.tile([128, DT, G, BL], BF16, tag="poT")
            wstep = 0

            # ---- phase D: attention, two samples per pair ----
            for p in range(BL // 2):
                hk, hkT = hk_cur, hkT_cur
                xkT_next = None
                if p < 3:
                    xkT_next = xkt_p.tile([128, 3, 512], BF16, tag="xkT")
                    for t in range(4):
                        xrow_tiles[("k", t)] = gather(idxk_sb, (p + 1) * 4 + t)

                for bi in range(2):
                    b = p * 2 + bi

                    # D5: logits.T [k, (g,q)] in PSUM: [128, 2, 256]
                    ps_l = lg_p.tile([128, 512], F32, tag="lg")
                    for kt in range(2):
                        for c in range(DT):
                            nc.tensor.matmul(
                                ps_l[:, kt * 256 : (kt + 1) * 256],
                                lhsT=hkT[
                                    :, c,
                                    bi * 256 + kt * 128 : bi * 256 + (kt + 1) * 128,
                                ],
                                rhs=(hqw_lo if b < 4 else hqw_hi)[
                                    :, c, b % 4],
                                start=(c == 0),
                                stop=(c == DT - 1),
                            )

                    # D6: E = exp(logits) bf16 (one op), zz sums (one op)
                    et = et_p.tile([128, 2, G * NQ], BF16, tag="et")
                    zz = zz_p.tile([128, 2, G], F32, tag="zz")
                    nc.scalar.activation(
                        out=et[:], in_=ps_l[:], func=AF.Exp)
                    nc.vector.tensor_reduce(
                        out=zz[:],
                        in_=et[:].rearrange("p t (g q) -> p t g q", g=G),
                        axis=AX.X,
                        op=OP.add,
                    )

                    # D8: u = hk.T @ E per 2 d-tiles; v = u * hq; vr = sum_q
                    vr_all = vr_p.tile([128, DT, G], F32, tag="vr")
                    for mp in range(4):
                        ps_u = up_p.tile([128, 512], F32, tag="up")
                        for mi in range(2):
                            m = mp * 2 + mi
                            for kt in range(2):
                                nc.tensor.matmul(
                                    ps_u[:, mi * 256 : (mi + 1) * 256],
                                    lhsT=hk[:, bi * 2 + kt, m * 128 : (m + 1) * 128],
                                    rhs=et[:, kt, :],
                                    start=(kt == 0),
                                    stop=(kt == 1),
                                )
                        v = v_p.tile([128, 2, G, NQ], BF16, tag="v")
                        nc.vector.tensor_tensor(
                            out=v[:],
                            in0=ps_u[:].rearrange("p (m g q) -> p m g q", m=2, g=G),
                            in1=hqT[
                                :, mp * 2 : mp * 2 + 2, None, b * NQ : (b + 1) * NQ
                            ].to_broadcast([128, 2, G, NQ]),
                            op=OP.mult,
                        )
                        nc.vector.tensor_reduce(
                            out=vr_all[:, mp * 2 : mp * 2 + 2, :], in_=v[:],
                            axis=AX.X, op=OP.add,
                        )

                    # D7 (late so the PE never waits on it): Z_g over
                    # k-partitions, then one fused pooled scale.
                    ps_z = mm_p.tile([128, 512], F32, tag="mm")
                    for kt in range(2):
                        nc.tensor.matmul(
                            ps_z[:1, :G],
                            lhsT=ones_sb[:],
                            rhs=zz[:, kt, :],
                            start=(kt == 0),
                            stop=(kt == 1),
                        )
                    zinv = zn_p.tile([1, G], F32, tag="zinv")
                    nc.vector.reciprocal(zinv[:1, :], ps_z[:1, :G])
                    zbro = zn_p.tile([128, G], F32, tag="zbro")
                    nc.gpsimd.partition_broadcast(zbro[:], zinv[:1, :], channels=128)
                    with nc.allow_low_precision(reason="bf16 pooled"):
                        nc.vector.tensor_tensor(
                            out=poT[:, :, :, b],
                            in0=vr_all[:],
                            in1=zbro[:, None, :].to_broadcast([128, DT, G]),
                            op=OP.mult,
                        )

                    if bi == 0:
                        if p < 3:
                            # next pair's X transposes mid-pair: their
                            # gathers land about now, and D2(p+1) needs
                            # xkT well before the pair ends
                            for t in range(4):
                                transpose_x(xrow_tiles.pop(("k", t)),
                                            xkT_next, t)
                        emit_weight_chunk(wstep); wstep += 1
                        emit_weight_chunk(wstep); wstep += 1

                emit_weight_chunk(wstep); wstep += 1
                if p < 3:
                    hk_cur, hkT_cur = project_k_pair(xkT_next)

            # attention pools (incl. all PSUM) are dead now
            actx.close()
            fctx = contextlib.ExitStack()
            fo_p = fctx.enter_context(tc.tile_pool(name="fo", bufs=2, space="PSUM"))

            # ---- phase F: out [8, 300] = pooled_flat @ Wout + bout ----
            ps_o = fo_p.tile([128, 512], F32, tag="fo")
            for g in range(G):
                for m in range(DT):
                    t = g * DT + m
                    nc.tensor.matmul(
                        ps_o[:BL, :N_OUT],
                        lhsT=poT[:, m, g, :],
                        rhs=wout_sb[:, t, :],
                        start=(t == 0),
                        stop=(t == NWOUT - 1),
                    )
            out_sb = consts.tile([BL, N_OUT], BF16, tag="out_sb")
            with nc.allow_low_precision(reason="bf16 out"):
                nc.vector.tensor_tensor(
                    out=out_sb[:], in0=ps_o[:BL, :N_OUT], in1=bout_sb[:], op=OP.add
                )

            # ---- phase G: sim + log_softmax (no max shift; sim is O(+-5)) --
            outT = consts.tile([128, 3, BL], BF16, tag="outT")
            for c, rows in enumerate(N_CHUNKS):
                psT = fo_p.tile([128, 128], BF16, tag="foT")
                nc.tensor.transpose(
                    psT[:rows, :BL],
                    out_sb[:, c * 128 : c * 128 + rows],
                    ident[:BL, :BL],
                )
                nc.scalar.activation(out=outT[:rows, c, :], in_=psT[:rows, :BL],
                                     func=AF.Copy)

            zs8 = consts.tile([BL, NA_CH], F32, tag="zs8")
            zs = consts.tile([BL, 1], F32, tag="zs")
            zsi = consts.tile([BL, 1], F32, tag="zsi")
            nlnz = consts.tile([BL, 1], F32, tag="nlnz")
            final_sb = consts.tile([BL, N_ANS], F32, tag="final")

            simp_tiles = []
            esc_p = ctx.enter_context(tc.tile_pool(name="esc", bufs=2))
            fctx.close()  # free F/outT PSUM banks before claiming all 8
            sim_p = ctx.enter_context(tc.tile_pool(name="simp", bufs=NA_CH,
                                                   space="PSUM"))
            for a in range(NA_CH):
                ps_s = sim_p.tile([128, NA_W], F32, tag="simp")
                for c, rows in enumerate(N_CHUNKS):
                    nc.tensor.matmul(
                        ps_s[:BL, :],
                        lhsT=outT[:rows, c, :],
                        rhs=glove_sb[:rows, c, a * NA_W : (a + 1) * NA_W],
                        start=(c == 0),
                        stop=(c == 2),
                    )
                esc = esc_p.tile([BL, NA_W], BF16, tag="esc")
                nc.scalar.activation(out=esc[:], in_=ps_s[:BL, :], func=AF.Exp)
                nc.vector.tensor_reduce(
                    out=zs8[:, a : a + 1], in_=esc[:], axis=AX.X, op=OP.add
                )
                simp_tiles.append(ps_s)

            nc.vector.tensor_reduce(out=zs[:], in_=zs8[:], axis=AX.X, op=OP.add)
            nc.vector.reciprocal(zsi[:], zs[:])
            nc.scalar.activation(out=nlnz[:], in_=zsi[:], func=AF.Ln)
            # final = sim - lnZ, 4 chunks on DVE + 4 on Act, then 2 DMAs
            for a in range(NA_CH):
                span = slice(a * NA_W, (a + 1) * NA_W)
                if a % 2 == 0:
                    nc.vector.tensor_scalar(
                        out=final_sb[:, span], in0=simp_tiles[a][:BL, :],
                        scalar1=nlnz[:], scalar2=None,
                        op0=OP.add,
                    )
                else:
                    nc.scalar.activation(
                        out=final_sb[:, span], in_=simp_tiles[a][:BL, :],
                        func=AF.Identity, bias=nlnz[:],
                    )
                if a == 3:
                    nc.sync.dma_start(out_d[:, : 4 * NA_W], final_sb[:, : 4 * NA_W])
            nc.sync.dma_start(out_d[:, 4 * NA_W :], final_sb[:, 4 * NA_W :])

    nc.compile()
    return nc


_NC = None


def _get_nc():
    global _NC
    if _NC is None:
        _NC = build_kernel()
    return _NC


def make_in_maps(inputs):
    import ml_dtypes

    bf = ml_dtypes.bfloat16
    he_q = np.asarray(inputs["he_ques"]).astype(np.int32)   # [64, 32]
    he_k = np.asarray(inputs["he_kg"]).astype(np.int32)     # [64, 256]
    emb0 = np.asarray(inputs["emb"], dtype=np.float32)
    emb = np.zeros((VOCAB, EA), dtype=bf)
    emb[:, :E] = emb0.astype(bf)
    emb[:, E] = np.ones((), dtype=bf)                       # bias column
    wq = np.zeros((EA, H), dtype=bf)
    wq[:E] = np.asarray(inputs["Wq"], np.float32).astype(bf)
    wq[E] = np.asarray(inputs["bq"], np.float32).astype(bf)
    wk = np.zeros((EA, H), dtype=bf)
    wk[:E] = np.asarray(inputs["Wk"], np.float32).astype(bf)
    wk[E] = np.asarray(inputs["bk"], np.float32).astype(bf)
    watt = np.ascontiguousarray(
        np.asarray(inputs["Watt"], np.float32).reshape(DT, 128, G)
        .transpose(1, 0, 2)).astype(bf)                     # [128, DT, G]
    wout = np.ascontiguousarray(
        np.asarray(inputs["Wout"], np.float32)).astype(bf)
    bout = np.ascontiguousarray(
        np.broadcast_to(np.asarray(inputs["bout"], np.float32), (BL, N_OUT)))
    glovet = np.ascontiguousarray(
        np.asarray(inputs["glove_cands"], np.float32).T).astype(bf)  # [300,4000]

    in_maps = []
    for i in range(N_CORES):
        iq = he_q[i * BL : (i + 1) * BL].reshape(-1)        # [256]
        ik = he_k[i * BL : (i + 1) * BL].reshape(-1)        # [2048]
        in_maps.append({
            "emb": emb,
            "idx_q": np.ascontiguousarray(iq.reshape(TQ_TILES, 128).T),
            "idx_k": np.ascontiguousarray(ik.reshape(TK_TILES, 128).T),
            "wq": wq,
            "wk": wk,
            "watt": watt,
            "wout": wout,
            "bout": bout,
            "glovet": glovet,
        })
    return in_maps


def kernel(**inputs) -> np.ndarray:
    nc = _get_nc()
    in_maps = make_in_maps(inputs)
    res = run_bass_kernel_spmd(nc, in_maps, list(range(N_CORES)))
    return np.concatenate(
        [np.asarray(res.results[i]["out"], np.float32) for i in range(N_CORES)],
        axis=0,
    )
